# revision 1
# baseline (speedup 1.0000x reference)
"""Trainium2 distributed kernel for the modular spiking-network module.

Model (reference semantics):
  1. 16 modular units, each a LIF recurrence over shared input spikes
     (T=100, N=1024) with per-unit input / recurrent [N,N] weights.
  2. Per-unit mean activity -> coordinator MLP -> sigmoid probs [16,16].
  3. Bernoulli routing matrix conn = (U42 < probs), U42 fixed uniform draws.
  4. routed = einsum('ij,itn->tjn', conn, outputs);
     applied = einsum('tjn,jnm->tjm', routed, unit_w);
     out = applied.mean(axis=1) + 1.5 * input_spikes.

Key structural fact exploited on-device: while no neuron has spiked, the
LIF dynamics are LINEAR, so the membrane trajectory is a causal filter of
the input currents:  v_dec(t) - V_LEAK = sum_{r<t} K[t,r] * C[r], with
C = spikes @ w_in.T and K a [T,T] constant kernel.  Spikes are then
z = (v_dec - V_TH > 0).  The device computes this exactly (it detects the
FIRST threshold crossing correctly even if one were to occur), counts all
spikes into a `zsum` output, and the host falls back to an exact
sequential evaluation in the (never observed) case zsum > 0.  With the
benchmark weight scales, max membrane drift is ~0.14 vs a 15.0 threshold
gap, so the linear path is exact and the routing/apply stages run on the
true all-zero spike tensors.

Sharding (8 cores): units are sharded 2-per-core for the LIF stage; the
[U,N] spike tensors + partial MLP activations are all-gathered; each core
then routes + applies for its 2 target units and a final all-reduce forms
the combined output.
"""

import ml_dtypes
import numpy as np

import concourse.mybir as mybir
import concourse.tile as tile
from concourse import bacc
from concourse.bass_utils import run_bass_kernel_spmd
from concourse.tile_rust import add_dep_helper

# ---------------------------------------------------------------- constants
T, U, N, H = 100, 16, 1024, 128
NCORES = 8
UPC = U // NCORES          # units per core
KC = N // 128              # 128-row chunks per [N,N] matrix
DT = 1e-3
TAU_MEM_INV = 1.0 / 20.0
TAU_SYN_INV = 1.0 / 10.0
V_LEAK = -70.0
V_TH = -55.0
V_RESET = -70.0
DIRECT_WEIGHT = 1.5
THRESH = np.float32(V_TH - V_LEAK)       # 15.0

ZT_FLAT = 128 * KC * T                   # one unit's z, [128, KC, T] flattened
PAYLOAD = UPC * ZT_FLAT + 512            # per-core AG payload (uint8 z; tail = 128 f32 h bitcast)
CCOLS = U * UPC                          # conn columns used per core (i, local j)

# jax.random.uniform(jax.random.key(42), (16, 16)) — fixed module constant.
_U42_HEX = (
    "d010183f4043e03ee8e9203f80a1013c80838e3eb0304c3f7c265a3f6868763f"
    "5c332d3f406be33dc8f2fd3e3c2c3b3f9042423e40201e3ea0c31b3dccd2ab3e"
    "3ea26d3f10c61c3f5039fc3eb6da3b3fe0bf413e108f1d3e201c183d049aa83e"
    "86ec6f3f2883183fe855ec3eca682b3fc0c18f3d04c5c93e36d00e3f5c50b53e"
    "62a8733f4a50223f00bcd53bc4c28c3ebcd8493f20dc563f66c5683f1ae2143f"
    "c434d93e56781c3f6890fb3e40573e3f50b3653ea01e5c3ec0142f3e80e2933d"
    "b015c23e38f8073ffc2e943e36f2513fec45663f4c29093f00d2af3eae696a3f"
    "7478113f48c5ce3ee8550b3fd06da63e8afa633fa27d023f1ce2823e0e66473f"
    "12b04b3f06e0533ffa44633f1610023f7c59813e94e9453ff6614f3fe66d5a3f"
    "a07c703fc464243f2093423d3062b93ede997d3f3ec83f3f00ec6d3e1890483e"
    "486f023e409cbc3c7c069e3eb8cf5a3faa59713f540c263fe09f053de8e2a93e"
    "a0ae6d3f661b1f3fa4ddf43ea687303f386d153ea0886f3df403b33eb271773f"
    "d89e2a3f605a8a3d70fbcd3e5892093f5082ae3e9ee66a3f6c93113fe00ace3e"
    "f82d0a3fe0f2a03edcde643f32980d3f8012be3eec2a7a3fa66f303fc8a8133e"
    "a0005e3dacdcbf3ea6cd7b3f808a333fc80d0c3e800b053c8068803e5673443f"
    "d8f04c3f36e95d3fcedb7f3fd6b93b3ff8e34d3e48d70b3e005bf73bd0be8f3e"
    "d2404c3fac81543f6601653fb60e063fe036803eaa2e4c3ffe46543f7c98643f"
    "343e053f8c9e8c3ed4af403fdc624d3fc0f9563f8ac5613fb4a30f3f5cdda63e"
    "befb6a3ff4d4193fc46fff3e700f333f18ff293e7091c43d6c93fa3efaf6363f"
    "d853063ee017753d30b0a53eeec4693f70fd1f3fd029e73e0c792b3fc017d53d"
    "dc2ff23e96723e3f88e2423e1892343e5000b03d8490c03e78de0c3f24efab3e"
    "daa8673fbc12033fb036943e70f7583f8e2e7d3f9e9b363f98d2073ec051723d"
    "8c0ea53edac4693f964e1f3f748ae43e6853283f50c6e33d1c55e83e1481243f"
    "40e3ba3c48f88e3e2a37423f6e9e483f24cd5d3ffc6c773f9a2a223f20880a3d"
    "345bbb3e4ca0773f40b9233fa05b383dfcc6b73e2a2f7b3f00b03a3f182a663e"
    "98c67f3ed01d4c3eb8af2b3e50adc93d9c6bfd3e3ccb313f50d83c3ec069963d"
    "507bd23ec8de1e3f9c3de23e96912e3fc07e8c3d006adf3e54c0133f9c4fd63e"
    "f6f71a3f40aff23e7c163e3f204d423ec06c363e806bbc3d485ec73ee2e90b3f"
    "e8cdb63e507e7a3f024d383f28d4703ec0e9533ed892153e8008cb3c044c803e"
    "deda4c3ff020553f80d2663f3a39013f08db9d3e2246513f00166e3fbeb6103f"
    "30f7db3eba7f173fc4eec43e3e65083fa886b83ee008743f029f243fc06eb63c"
    "68da8f3e4a5e433f3c384a3f6af7583f426b7d3f8c54363f78ae003ec0634d3d"
)


def _u42() -> np.ndarray:
    return np.frombuffer(bytes.fromhex(_U42_HEX), dtype=np.float32).reshape(U, U)


def _kmat() -> np.ndarray:
    """K[t, r] = m * (a^(t-r) - b^(t-r)) / (a - b) for r < t, else 0 (f32)."""
    m = DT * TAU_MEM_INV
    a = 1.0 - m
    b = 1.0 - DT * TAU_SYN_INV
    d = np.arange(T, dtype=np.float64)
    coef = np.zeros(T, np.float64)
    coef[1:] = m * (a ** d[1:] - b ** d[1:]) / (a - b)
    idx = np.arange(T)
    K = coef[np.clip(idx[:, None] - idx[None, :], 0, T - 1)]
    K[idx[:, None] <= idx[None, :]] = 0.0
    return K.astype(np.float32)


# ---------------------------------------------------------------- graph
_GRAPH_CACHE = {}


def _build_graph():
    if "nc" in _GRAPH_CACHE:
        return _GRAPH_CACHE["nc"]

    f32 = mybir.dt.float32
    bf16 = mybir.dt.bfloat16
    fp8 = mybir.dt.float8e4
    u8 = mybir.dt.uint8
    Alu = mybir.AluOpType
    Act = mybir.ActivationFunctionType

    nc = bacc.Bacc("TRN2", target_bir_lowering=False, debug=False,
                   num_devices=NCORES)

    # I/O (per-core shards / replicas)
    sp_ext = nc.dram_tensor("sp", [T, N], f32, kind="ExternalInput").ap()
    spt_ext = nc.dram_tensor("spt", [128, KC * 128], fp8, kind="ExternalInput").ap()
    wint_ext = nc.dram_tensor("wint", [128, UPC * KC * N], fp8,
                              kind="ExternalInput").ap()
    uw_ext = nc.dram_tensor("uw", [128, UPC * KC * N], bf16,
                            kind="ExternalInput").ap()
    ktm_ext = nc.dram_tensor("ktm", [128, T], f32, kind="ExternalInput").ap()
    cw1t_ext = nc.dram_tensor("cw1t", [128, UPC * KC * H], bf16,
                              kind="ExternalInput").ap()
    cw2t_ext = nc.dram_tensor("cw2t", [H, CCOLS], f32, kind="ExternalInput").ap()
    cb1_ext = nc.dram_tensor("cb1c", [H, 1], f32, kind="ExternalInput").ap()
    u42lc_ext = nc.dram_tensor("u42lc", [128, CCOLS], f32,
                               kind="ExternalInput").ap()
    out_ext = nc.dram_tensor("out", [T * N // NCORES, 1], f32,
                             kind="ExternalOutput").ap()
    zsum_ext = nc.dram_tensor("zsum", [1, 1], f32, kind="ExternalOutput").ap()

    with tile.TileContext(nc) as tc:
        with (
            tc.tile_pool(name="wpool", bufs=24) as wpool,      # weight chunks
            tc.tile_pool(name="zpool", bufs=1) as zpool,       # gathered spikes
            tc.tile_pool(name="work", bufs=1) as work,         # misc persistents
            tc.tile_pool(name="rbuf", bufs=1) as rbuf,         # routed ping-pong
            tc.tile_pool(name="psA", bufs=3, space="PSUM") as psA,
            tc.tile_pool(name="psB", bufs=2, space="PSUM") as psB,
            tc.tile_pool(name="psC", bufs=1, space="PSUM") as psC,
            tc.tile_pool(name="dram", bufs=1, space="DRAM") as dram,
        ):
            # ---------- persistent SBUF
            spt = work.tile([128, KC, 128], fp8)
            nc.sync.dma_start(spt[:], spt_ext.rearrange("p (k t) -> p k t", t=128))
            sp_sb = work.tile([T, N], f32)
            sp_dma = nc.sync.dma_start(sp_sb[:], sp_ext)
            ktm = work.tile([128, T], f32)
            nc.sync.dma_start(ktm[:], ktm_ext)
            cw1t = work.tile([128, UPC * KC, H], bf16)
            nc.sync.dma_start(cw1t[:],
                              cw1t_ext.rearrange("p (c h) -> p c h", h=H))
            cw2t = work.tile([H, CCOLS], f32)
            nc.sync.dma_start(cw2t[:], cw2t_ext)
            cb1c = work.tile([H, 1], f32)
            nc.sync.dma_start(cb1c[:], cb1_ext)
            u42lc = work.tile([128, CCOLS], f32)
            nc.sync.dma_start(u42lc[:], u42lc_ext)
            ones_col = work.tile([128, 1], f32)
            nc.vector.memset(ones_col[:], 1.0)


            # ---------- stage 1: C_u = spikes @ w_in[u].T   (natural [t, m])
            # wint chunk (u, kc) = w_in[u].T[kc*128:(kc+1)*128, :]  -> [128, N]
            wchunks = {}
            for u in range(UPC):
                for kcp in range(KC // 2):
                    w = wpool.tile([128, 2, N], fp8, tag="wchunk",
                                   name=f"win_{u}_{kcp}")
                    nc.sync.dma_start(
                        w[:], wint_ext[:, (u * KC + 2 * kcp) * N:
                                       (u * KC + 2 * kcp + 2) * N]
                        .rearrange("p (k n) -> p k n", k=2))
                    wchunks[(u, kcp)] = w

            c_tiles = []
            for u in range(UPC):
                cu = work.tile([128, N], f32, name=f"c_{u}")
                nc.vector.memset(cu[:], 0.0)        # zero-pad rows T..127
                for mh in range(2):
                    pc = psA.tile([128, 512], f32, tag="ps", name=f"psc_{u}_{mh}")
                    for kcp in range(KC // 2):
                        # fp8 DoubleRow: 256-deep contraction per pass
                        nc.tensor.matmul(
                            pc[:, :],
                            spt[:, 2 * kcp:2 * kcp + 2, :],      # [128, 2, 128]
                            wchunks[(u, kcp)][:, :, mh * 512:(mh + 1) * 512],
                            start=(kcp == 0), stop=(kcp == KC // 2 - 1),
                            perf_mode=mybir.MatmulPerfMode.DoubleRow)
                    nc.vector.tensor_copy(cu[:T, mh * 512:(mh + 1) * 512],
                                          pc[:T, :])
                c_tiles.append(cu)

            # ---------- stage 2: vd - V_LEAK = K @ C ; z = (. > 15)
            zt_loc = []
            act_t = []
            for u in range(UPC):
                zu = work.tile([128, KC, T], u8, name=f"z_{u}")
                for b in range(2):
                    # pack 4 stage-2 outputs per PSUM bank -> 1 threshold op
                    pv = psA.tile([128, 512], f32, tag="ps", name=f"psv_{u}_{b}")
                    for k in range(4):
                        nt = b * 4 + k
                        nc.tensor.matmul(pv[:, k * T:(k + 1) * T],
                                         c_tiles[u][:, nt * 128:(nt + 1) * 128],
                                         ktm[:], start=True, stop=True)
                    nc.vector.tensor_scalar(
                        zu[:, b * 4:(b + 1) * 4, :],
                        pv[:, :4 * T].rearrange("p (k t) -> p k t", t=T),
                        float(THRESH), None, Alu.is_gt)
                zt_loc.append(zu)
                au = work.tile([128, KC], bf16, name=f"act_{u}")
                with nc.allow_low_precision(
                        reason="spike counts <=100 are exact in bf16"):
                    nc.vector.tensor_reduce(au[:], zu[:], mybir.AxisListType.X,
                                            Alu.add)
                act_t.append(au)

            # ---------- MLP layer-1 partial:  hpart = sum_k act_raw[k] cw1[:,k]
            ph = psC.tile([H, 1], f32, tag="ph")
            for c in range(UPC * KC):
                u, nt = divmod(c, KC)
                nc.tensor.matmul(ph[:], cw1t[:, c, :], act_t[u][:, nt:nt + 1],
                                 start=(c == 0), stop=(c == UPC * KC - 1))
            hpart = work.tile([H, 1], f32)
            nc.vector.tensor_copy(hpart[:], ph[:])

            # ---------- all-gather  (z of local units + raw h partial)
            cc_in = dram.tile([PAYLOAD], u8)
            payload_dmas = []
            for u in range(UPC):
                d = nc.sync.dma_start(
                    cc_in[u * ZT_FLAT:(u + 1) * ZT_FLAT]
                    .rearrange("(p k t) -> p k t", p=128, k=KC),
                    zt_loc[u][:])
                payload_dmas.append(d)
            payload_dmas.append(nc.sync.dma_start(
                cc_in[UPC * ZT_FLAT:PAYLOAD].bitcast(f32)
                .rearrange("(p o) -> p o", p=H),
                hpart[:]))
            add_dep_helper(sp_dma.ins, payload_dmas[-1].ins,
                           reason="defer residual spikes load past AG payload")
            cc_out = dram.tile([NCORES, PAYLOAD], u8, addr_space="Shared")
            nc.gpsimd.collective_compute(
                "AllGather", Alu.bypass,
                ins=[cc_in[:].opt()], outs=[cc_out[:].opt()],
                replica_groups=[list(range(NCORES))])

            # ---------- gathered spikes + h reduction
            # Two tiles (one per local-unit slot) so the routed chain can start
            # on the first half while the second half is still landing.
            # Slot tile uu holds global units i with i % UPC == uu, at index
            # i // UPC.
            hparts = work.tile([H, NCORES], f32)
            with nc.allow_non_contiguous_dma(
                    reason="4KB gather of 8 partial-h columns"):
                nc.sync.dma_start(hparts[:],
                                  cc_out[:, UPC * ZT_FLAT:PAYLOAD]
                                  .bitcast(f32).rearrange("c p -> p c"))

            zt_half = []
            rb_dmas = []
            G = NCORES // 2
            for uu in range(UPC):
                zh = zpool.tile([128, NCORES, KC * T], u8, name=f"zth_{uu}")
                for g in range(2):
                    d = nc.sync.dma_start(
                        zh[:, g * G:(g + 1) * G, :],
                        cc_out[g * G:(g + 1) * G,
                               uu * ZT_FLAT:(uu + 1) * ZT_FLAT]
                        .rearrange("c (p f) -> p c f", p=128))
                    rb_dmas.append(d)
                zt_half.append(zh)

            def zt_slot(i):
                return zt_half[i % UPC][:, i // UPC, :]
            hsum = work.tile([H, 1], f32)
            nc.vector.tensor_reduce(hsum[:], hparts[:], mybir.AxisListType.X,
                                    Alu.add)
            h_pre = work.tile([H, 1], f32)
            nc.vector.scalar_tensor_tensor(h_pre[:], hsum[:], 1.0 / T,
                                           cb1c[:], Alu.mult, Alu.add)
            h_col = work.tile([H, 1], f32)
            nc.vector.tensor_scalar(h_col[:], h_pre[:], 0.0, None, Alu.max)

            # ---------- conn, replicated on all 128 partitions
            h_b = work.tile([H, H], f32)
            nc.vector.tensor_copy(h_b[:], h_col[:].to_broadcast([H, H]))
            pl = psC.tile([128, CCOLS], f32, tag="pl")
            nc.tensor.matmul(pl[:], h_b[:], cw2t[:], start=True, stop=True)
            # conn = (u42 < sigmoid(pl + cb2)) == (logit(u42) - cb2 < pl)
            conn_b = work.tile([128, CCOLS], f32)
            nc.vector.tensor_tensor(conn_b[:], u42lc[:], pl[:], Alu.is_lt)

            # ---------- unit_w chunks for the local target units.
            # Deferred behind the all-gather payload DMAs so the 8MB load
            # rides the collective window instead of starving the stage-1
            # weight streams.
            uchunks = {}
            for j in range(UPC):
                for nt in range(KC):
                    w = wpool.tile([128, N], bf16, tag="wchunk",
                                   name=f"uw_{j}_{nt}")
                    d = nc.sync.dma_start(
                        w[:], uw_ext[:, (j * KC + nt) * N:(j * KC + nt + 1) * N])
                    add_dep_helper(d.ins, payload_dmas[-1].ins,
                                   reason="defer unit_w load past AG payload")
                    uchunks[(j, nt)] = w

            # ---------- routed (transposed layout) for local targets
            # The SPMD graph is identical on every core, so the per-core
            # target-unit offset enters through DATA: cw2t/cb2b/u42b are fed
            # pre-permuted per core so that conn_b column [i*UPC + j] holds
            # the (source i, target 2*core+j) entry.  (See host prep.)
            # Chain consumes the first-half units (even i) before the odd ones
            # so it can start as soon as the first readback DMA lands.
            i_order = [i for i in range(U) if i % UPC == 0] + \
                      [i for i in range(U) if i % UPC != 0]
            # 16 fast multiplies (tensor_scalar, 4x bf16) into a scratch
            # [128, 16, 800], then an in-place binary tree of wide
            # tensor_tensor adds (2x bf16).  Slot q holds global unit
            # i = (q % NCORES) * UPC + q // NCORES.
            r_final = []
            for j in range(UPC):
                tmp = rbuf.tile([128, U, KC * T], bf16, tag=f"tmp{j}",
                                name=f"tmp{j}")
                for q in [q for uu in range(UPC) for g in range(2)
                          for q in range(uu * NCORES + g * G,
                                         uu * NCORES + (g + 1) * G)]:
                    i = (q % NCORES) * UPC + q // NCORES
                    col = conn_b[:, i * UPC + j:i * UPC + j + 1]
                    on_dve = (q % 4 != 3) if j == 0 else (q % 2 == 0)
                    if on_dve:
                        nc.vector.tensor_scalar(tmp[:, q, :], zt_slot(i),
                                                col, None, Alu.mult)
                    else:
                        # idle ScalarE helps: out = Copy(in * scale)
                        nc.scalar.activation(tmp[:, q, :], zt_slot(i),
                                             Act.Copy, scale=col)
                half = U // 2
                while half >= 1:
                    nc.vector.tensor_tensor(tmp[:, :half, :], tmp[:, :half, :],
                                            tmp[:, half:2 * half, :], Alu.add)
                    half //= 2
                r_final.append(tmp[:, 0, :])

            # ---------- applied + local combine.
            # unit_w is pre-scaled by 1/U on the host, and every core adds
            # (DIRECT_WEIGHT / NCORES) * spikes, so the final AllReduce output
            # IS the module output (8 * 0.1875 = 1.5 exactly in fp32).
            comb = work.tile([T, N], f32)
            pos = [psB.tile([128, 512], f32, tag="po", name=f"po_{mh}")
                   for mh in range(2)]
            for j in range(UPC):
                for nt in range(KC):
                    for mh in range(2):
                        nc.tensor.matmul(
                            pos[mh][:T, :],
                            r_final[j][:, nt * T:(nt + 1) * T],
                            uchunks[(j, nt)][:, mh * 512:(mh + 1) * 512],
                            start=((j, nt) == (0, 0)),
                            stop=((j, nt) == (UPC - 1, KC - 1)))
            comb_ops = []
            for mh in range(2):
                op = nc.vector.scalar_tensor_tensor(
                    comb[:, mh * 512:(mh + 1) * 512],
                    sp_sb[:, mh * 512:(mh + 1) * 512],
                    DIRECT_WEIGHT / NCORES, pos[mh][:T, :], Alu.mult, Alu.add)
                comb_ops.append(op)

            # ---------- zsum (spike counter, drives the host fallback).
            # Scheduled after `comb` so the big reduce rides the AllReduce
            # window instead of blocking the routed chain.
            zred = work.tile([128, UPC], f32)
            for uu in range(UPC):
                op = nc.vector.tensor_reduce(zred[:, uu:uu + 1], zt_half[uu][:],
                                             mybir.AxisListType.XY, Alu.add)
                add_dep_helper(op.ins, comb_ops[-1].ins,
                               reason="zsum reduce off the critical path")
            zred_tot = work.tile([128, 1], f32)
            nc.vector.tensor_tensor(zred_tot[:], zred[:, 0:1], zred[:, 1:2],
                                    Alu.add)
            pz = psC.tile([1, 1], f32, tag="pz")
            nc.tensor.matmul(pz[:], zred_tot[:], ones_col[:], start=True,
                             stop=True)
            zs_sb = work.tile([1, 1], f32)
            nc.vector.tensor_copy(zs_sb[:], pz[:])
            nc.sync.dma_start(zsum_ext, zs_sb[:])

            # ---------- final reduce-scatter; the host concatenates the 8
            # shards (pure layout).  Residual + 1/U scale are already folded.
            rs_in = dram.tile([T * N], f32)
            nc.sync.dma_start(rs_in[:].rearrange("(t n) -> t n", t=T), comb[:])
            rs_out = dram.tile([T * N // NCORES], f32)
            nc.gpsimd.collective_compute(
                "ReduceScatter", Alu.add,
                ins=[rs_in[:].opt()], outs=[rs_out[:].opt()],
                replica_groups=[list(range(NCORES))])
            nc.sync.dma_start(out_ext, rs_out[:].rearrange("(a b) -> a b", b=1))

    nc.compile()
    _GRAPH_CACHE["nc"] = nc
    return nc


# ---------------------------------------------------------------- host prep
def _prep_in_maps(sp, w_in, unit_w, cw1, cb1, cw2, cb2):
    K32 = _kmat()
    ktm = np.zeros((128, T), np.float32)
    ktm[:T, :] = K32.T                       # ktm[r, t] = K[t, r]

    spt3 = np.zeros((128, KC, 128), np.float32)
    spt3[:, :, :T] = sp.T.reshape(KC, 128, T).transpose(1, 0, 2)
    spt = np.ascontiguousarray(
        spt3.reshape(128, KC * 128).astype(ml_dtypes.float8_e4m3fn))

    u42 = _u42()
    cw2t_base = np.ascontiguousarray(cw2.T)            # [H, 256], col l = i*U+j
    cb2_row = cb2.reshape(U * U)
    u42_row = u42.reshape(U * U)

    in_maps = []
    for c in range(NCORES):
        us = [UPC * c + u for u in range(UPC)]
        wint = np.ascontiguousarray(
            np.stack([w_in[g].T.reshape(KC, 128, N) for g in us])
            .transpose(2, 0, 1, 3).reshape(128, UPC * KC * N)
            .astype(ml_dtypes.float8_e4m3fn))
        uw = np.ascontiguousarray(
            (np.stack([unit_w[g].reshape(KC, 128, N) for g in us])
             .transpose(2, 0, 1, 3).reshape(128, UPC * KC * N)
             * np.float32(1.0 / U)).astype(ml_dtypes.bfloat16))
        base = c * UPC * N
        cw1t = np.ascontiguousarray(
            cw1[:, base:base + UPC * N].T.reshape(UPC * KC, 128, H)
            .transpose(1, 0, 2).reshape(128, UPC * KC * H)
            .astype(ml_dtypes.bfloat16))
        # conn column remap: graph reads column i*UPC + j for target 2c+j.
        perm = np.array([i * U + (UPC * c + j)
                         for i in range(U) for j in range(UPC)])
        # graph column order is (i, j); map graph col q=(i*UPC+j) -> flat perm
        cw2t = np.ascontiguousarray(cw2t_base[:, perm])
        lc = (np.log(u42_row.astype(np.float64) /
                     (1.0 - u42_row.astype(np.float64)))
              - cb2_row.astype(np.float64)).astype(np.float32)
        u42lcb = np.ascontiguousarray(np.broadcast_to(lc[perm], (128, CCOLS)))
        in_maps.append({
            "sp": sp, "spt": spt, "wint": wint, "uw": uw, "ktm": ktm,
            "cw1t": cw1t, "cw2t": cw2t, "cb1c": cb1.reshape(H, 1),
            "u42lc": u42lcb,
        })
    return in_maps


# ---------------------------------------------------------------- fallback
def _reference_host(sp, w_in, w_rec, unit_w, cw1, cb1, cw2, cb2):
    """Exact sequential evaluation (used only if any spike fires)."""
    m = np.float32(DT * TAU_MEM_INV)
    bsyn = np.float32(1.0 - DT * TAU_SYN_INV)
    outs = np.zeros((U, T, N), np.float32)
    for uu in range(U):
        z = np.zeros(N, np.float32)
        v = np.full(N, V_LEAK, np.float32)
        i = np.zeros(N, np.float32)
        for t in range(T):
            vd = v + m * ((V_LEAK - v) + i)
            idec = i * bsyn
            zn = (vd - V_TH > 0).astype(np.float32)
            vn = (1 - zn) * vd + zn * V_RESET
            i = idec + sp[t] @ w_in[uu].T + z @ w_rec[uu].T
            z, v = zn, vn
            outs[uu, t] = zn
    act = outs.mean(axis=1)
    h = np.maximum(act.reshape(-1) @ cw1.T + cb1, 0).astype(np.float32)
    probs = (1.0 / (1.0 + np.exp(-(h @ cw2.T + cb2)))).reshape(U, U)
    conn = (_u42() < probs).astype(np.float32)
    routed = np.einsum('ij,itn->tjn', conn, outs)
    applied = np.einsum('tjn,jnm->tjm', routed, unit_w)
    return (applied.mean(axis=1) + DIRECT_WEIGHT * sp).astype(np.float32)


# ---------------------------------------------------------------- entry
def kernel(input_spikes, w_in, w_rec, unit_w, cw1, cb1, cw2, cb2,
           **_unused):
    sp = np.ascontiguousarray(np.asarray(input_spikes, np.float32))
    w_in = np.asarray(w_in, np.float32)
    w_rec = np.asarray(w_rec, np.float32)
    unit_w = np.asarray(unit_w, np.float32)
    cw1 = np.asarray(cw1, np.float32)
    cb1 = np.ascontiguousarray(np.asarray(cb1, np.float32))
    cw2 = np.asarray(cw2, np.float32)
    cb2 = np.asarray(cb2, np.float32)

    nc = _build_graph()
    in_maps = _prep_in_maps(sp, w_in, unit_w, cw1, cb1, cw2, cb2)
    res = run_bass_kernel_spmd(nc, in_maps, core_ids=list(range(NCORES)))
    out = np.concatenate(
        [np.asarray(res.results[c]["out"], np.float32).reshape(-1)
         for c in range(NCORES)]).reshape(T, N)
    zsum = float(np.asarray(res.results[0]["zsum"]).reshape(-1)[0])
    if zsum != 0.0:
        # A spike fired: the linearized fast path is invalid -> exact host
        # evaluation (never hit with the benchmark weight scales).
        return _reference_host(sp, w_in, w_rec, unit_w, cw1, cb1, cw2, cb2)
    return out


if __name__ == "__main__":
    d = np.load("inputs.npz")
    got = kernel(**{k: d[k] for k in d.files})
    ref = np.load("golden.npy")
    err = np.abs(got - ref).max()
    denom = max(np.abs(ref).max(), 1e-9)
    print("abs err:", err, "rel:", err / denom)



# revision 2
# speedup vs baseline: 5.1962x; 5.1962x over previous
"""Trainium2 distributed kernel for the modular spiking-network module.

Model (reference semantics):
  1. 16 modular units, each a LIF recurrence over shared input spikes
     (T=100, N=1024) with per-unit input / recurrent [N,N] weights.
  2. Per-unit mean activity -> coordinator MLP -> sigmoid probs [16,16].
  3. Bernoulli routing matrix conn = (U42 < probs), U42 fixed uniform draws.
  4. routed = einsum('ij,itn->tjn', conn, outputs);
     applied = einsum('tjn,jnm->tjm', routed, unit_w);
     out = applied.mean(axis=1) + 1.5 * input_spikes.

Key structural facts exploited on-device:

  (a) While no neuron has spiked, the LIF dynamics are LINEAR, so the
      membrane drift is a causal filter of the input currents:
      v_dec(t) - V_LEAK = (K @ sp @ w_in[u].T)[t], with K a constant
      [T,T] kernel.  Folding Ksp = K @ sp on the host (tiny [T,N]
      precompute), the whole per-unit trajectory is ONE matmul
      drift_u = Ksp @ w_in[u].T, and a spike exists iff
      max(drift_u) > V_TH - V_LEAK = 15.

  (b) If no spike fires in ANY unit, every downstream stage is exactly
      zero: outputs == 0 -> activity == 0 -> routed == 0 -> applied == 0,
      and the module output is exactly DIRECT_WEIGHT * input_spikes.

So the device does exactly the irreducible work: it streams all 16
units' [N,N] input weights (fp8, units sharded 2-per-core), computes
each unit's full drift trajectory, and reduces it to a per-core
max-drift scalar.  Each core also emits its 128-column shard of the
residual output 1.5 * input_spikes.  The host ORs the 8 max-drift flags;
if any unit would spike (never observed with the benchmark weight
scales: max drift ~0.14 vs a 15.0 threshold gap, and we flag at 7.5 to
absorb fp8 rounding), it falls back to an exact sequential evaluation.
"""

import ml_dtypes
import numpy as np

import concourse.mybir as mybir
import concourse.tile as tile
from concourse import bacc
from concourse.bass_utils import run_bass_kernel_spmd

# ---------------------------------------------------------------- constants
T, U, N, H = 100, 16, 1024, 128
NCORES = 8
UPC = U // NCORES          # units per core
KC = N // 128              # 128-row chunks per [N,N] matrix
DT = 1e-3
TAU_MEM_INV = 1.0 / 20.0
TAU_SYN_INV = 1.0 / 10.0
V_LEAK = -70.0
V_TH = -55.0
V_RESET = -70.0
DIRECT_WEIGHT = 1.5
THRESH = np.float32(V_TH - V_LEAK)       # 15.0

# fp8 scaling: keep both operands well inside e4m3's normal range so the
# detection matmul loses no low-magnitude rows to denormal flush.
KSP_SCALE = 64.0           # Ksp entries <= ~0.25  -> <= 16
W_SCALE = 16.0             # |w_in| <= 1/32        -> <= 0.5
# device drift is scaled by KSP_SCALE*W_SCALE; flag at half the true
# threshold so fp8 rounding can never hide a real crossing.
DET_TH = 0.5 * float(THRESH) * KSP_SCALE * W_SCALE   # 7680.0

# jax.random.uniform(jax.random.key(42), (16, 16)) — fixed module constant
# (used only by the exact host fallback).
_U42_HEX = (
    "d010183f4043e03ee8e9203f80a1013c80838e3eb0304c3f7c265a3f6868763f"
    "5c332d3f406be33dc8f2fd3e3c2c3b3f9042423e40201e3ea0c31b3dccd2ab3e"
    "3ea26d3f10c61c3f5039fc3eb6da3b3fe0bf413e108f1d3e201c183d049aa83e"
    "86ec6f3f2883183fe855ec3eca682b3fc0c18f3d04c5c93e36d00e3f5c50b53e"
    "62a8733f4a50223f00bcd53bc4c28c3ebcd8493f20dc563f66c5683f1ae2143f"
    "c434d93e56781c3f6890fb3e40573e3f50b3653ea01e5c3ec0142f3e80e2933d"
    "b015c23e38f8073ffc2e943e36f2513fec45663f4c29093f00d2af3eae696a3f"
    "7478113f48c5ce3ee8550b3fd06da63e8afa633fa27d023f1ce2823e0e66473f"
    "12b04b3f06e0533ffa44633f1610023f7c59813e94e9453ff6614f3fe66d5a3f"
    "a07c703fc464243f2093423d3062b93ede997d3f3ec83f3f00ec6d3e1890483e"
    "486f023e409cbc3c7c069e3eb8cf5a3faa59713f540c263fe09f053de8e2a93e"
    "a0ae6d3f661b1f3fa4ddf43ea687303f386d153ea0886f3df403b33eb271773f"
    "d89e2a3f605a8a3d70fbcd3e5892093f5082ae3e9ee66a3f6c93113fe00ace3e"
    "f82d0a3fe0f2a03edcde643f32980d3f8012be3eec2a7a3fa66f303fc8a8133e"
    "a0005e3dacdcbf3ea6cd7b3f808a333fc80d0c3e800b053c8068803e5673443f"
    "d8f04c3f36e95d3fcedb7f3fd6b93b3ff8e34d3e48d70b3e005bf73bd0be8f3e"
    "d2404c3fac81543f6601653fb60e063fe036803eaa2e4c3ffe46543f7c98643f"
    "343e053f8c9e8c3ed4af403fdc624d3fc0f9563f8ac5613fb4a30f3f5cdda63e"
    "befb6a3ff4d4193fc46fff3e700f333f18ff293e7091c43d6c93fa3efaf6363f"
    "d853063ee017753d30b0a53eeec4693f70fd1f3fd029e73e0c792b3fc017d53d"
    "dc2ff23e96723e3f88e2423e1892343e5000b03d8490c03e78de0c3f24efab3e"
    "daa8673fbc12033fb036943e70f7583f8e2e7d3f9e9b363f98d2073ec051723d"
    "8c0ea53edac4693f964e1f3f748ae43e6853283f50c6e33d1c55e83e1481243f"
    "40e3ba3c48f88e3e2a37423f6e9e483f24cd5d3ffc6c773f9a2a223f20880a3d"
    "345bbb3e4ca0773f40b9233fa05b383dfcc6b73e2a2f7b3f00b03a3f182a663e"
    "98c67f3ed01d4c3eb8af2b3e50adc93d9c6bfd3e3ccb313f50d83c3ec069963d"
    "507bd23ec8de1e3f9c3de23e96912e3fc07e8c3d006adf3e54c0133f9c4fd63e"
    "f6f71a3f40aff23e7c163e3f204d423ec06c363e806bbc3d485ec73ee2e90b3f"
    "e8cdb63e507e7a3f024d383f28d4703ec0e9533ed892153e8008cb3c044c803e"
    "deda4c3ff020553f80d2663f3a39013f08db9d3e2246513f00166e3fbeb6103f"
    "30f7db3eba7f173fc4eec43e3e65083fa886b83ee008743f029f243fc06eb63c"
    "68da8f3e4a5e433f3c384a3f6af7583f426b7d3f8c54363f78ae003ec0634d3d"
)


def _u42() -> np.ndarray:
    return np.frombuffer(bytes.fromhex(_U42_HEX), dtype=np.float32).reshape(U, U)


def _kmat() -> np.ndarray:
    """K[t, r] = m * (a^(t-r) - b^(t-r)) / (a - b) for r < t, else 0 (f32)."""
    m = DT * TAU_MEM_INV
    a = 1.0 - m
    b = 1.0 - DT * TAU_SYN_INV
    d = np.arange(T, dtype=np.float64)
    coef = np.zeros(T, np.float64)
    coef[1:] = m * (a ** d[1:] - b ** d[1:]) / (a - b)
    idx = np.arange(T)
    K = coef[np.clip(idx[:, None] - idx[None, :], 0, T - 1)]
    K[idx[:, None] <= idx[None, :]] = 0.0
    return K.astype(np.float32)


# ---------------------------------------------------------------- graph
_GRAPH_CACHE = {}


def _build_graph():
    if "nc" in _GRAPH_CACHE:
        return _GRAPH_CACHE["nc"]

    f32 = mybir.dt.float32
    fp8 = mybir.dt.float8e4
    Alu = mybir.AluOpType

    nc = bacc.Bacc("TRN2", target_bir_lowering=False, debug=False,
                   num_devices=NCORES)

    # I/O (per-core shards / replicas)
    kspt_ext = nc.dram_tensor("kspt", [128, KC * 128], fp8,
                              kind="ExternalInput").ap()
    wint_ext = nc.dram_tensor("wint", [128, UPC * KC * N], fp8,
                              kind="ExternalInput").ap()
    spc_ext = nc.dram_tensor("spc", [T, 128], f32, kind="ExternalInput").ap()
    out_ext = nc.dram_tensor("out", [T, 128], f32, kind="ExternalOutput").ap()
    zsum_ext = nc.dram_tensor("zsum", [1, 1], f32, kind="ExternalOutput").ap()

    with tile.TileContext(nc) as tc:
        with (
            tc.tile_pool(name="wpool", bufs=8) as wpool,      # weight chunks
            tc.tile_pool(name="work", bufs=1) as work,        # persistents
            tc.tile_pool(name="ps", bufs=4, space="PSUM") as ps,
        ):
            # ---------- persistent SBUF
            kspt = work.tile([128, KC, 128], fp8)
            nc.sync.dma_start(kspt[:],
                              kspt_ext.rearrange("p (k t) -> p k t", t=128))
            spc = work.tile([T, 128], f32)
            nc.sync.dma_start(spc[:], spc_ext)

            # stream both units' [N,N] transposed input weights (fp8)
            wch = {}
            for u in range(UPC):
                for kcp in range(KC // 2):
                    w = wpool.tile([128, 2, N], fp8, tag="wchunk",
                                   name=f"win_{u}_{kcp}")
                    nc.sync.dma_start(
                        w[:], wint_ext[:, (u * KC + 2 * kcp) * N:
                                       (u * KC + 2 * kcp + 2) * N]
                        .rearrange("p (k n) -> p k n", k=2))
                    wch[(u, kcp)] = w

            # ---------- residual output shard: 1.5 * spikes (independent)
            outc = work.tile([T, 128], f32)
            nc.vector.tensor_scalar(outc[:], spc[:], DIRECT_WEIGHT, None,
                                    Alu.mult)
            nc.sync.dma_start(out_ext, outc[:])

            # ---------- spike detection: drift_u = Ksp @ w_in[u].T
            # one fp8 DoubleRow matmul chain per (unit, 512-col half);
            # reduce each PSUM tile to a per-timestep max as it completes.
            mxc = work.tile([128, UPC * 2], f32)
            for u in range(UPC):
                for mh in range(2):
                    pv = ps.tile([128, 512], f32, tag="ps",
                                 name=f"pv_{u}_{mh}")
                    for kcp in range(KC // 2):
                        nc.tensor.matmul(
                            pv[:, :],
                            kspt[:, 2 * kcp:2 * kcp + 2, :],
                            wch[(u, kcp)][:, :, mh * 512:(mh + 1) * 512],
                            start=(kcp == 0), stop=(kcp == KC // 2 - 1),
                            perf_mode=mybir.MatmulPerfMode.DoubleRow)
                    g = 2 * u + mh
                    nc.vector.tensor_reduce(mxc[:, g:g + 1], pv[:],
                                            mybir.AxisListType.X, Alu.max)

            # cross-partition max -> [1,1] scalar drift flag for the host
            zs = work.tile([1, 1], f32)
            nc.gpsimd.tensor_reduce(zs[:], mxc[:], mybir.AxisListType.XYZWC,
                                    Alu.max)
            nc.sync.dma_start(zsum_ext, zs[:])

    nc.compile()
    _GRAPH_CACHE["nc"] = nc
    return nc


# ---------------------------------------------------------------- host prep
def _prep_in_maps(sp, w_in):
    K32 = _kmat()
    ksp = (K32.astype(np.float64) @ sp.astype(np.float64)) * KSP_SCALE  # [T,N]
    kspt3 = np.zeros((128, KC, 128), np.float32)
    kspt3[:, :, :T] = ksp.T.reshape(KC, 128, T).transpose(1, 0, 2)
    kspt = np.ascontiguousarray(
        kspt3.reshape(128, KC * 128).astype(ml_dtypes.float8_e4m3fn))

    in_maps = []
    for c in range(NCORES):
        us = [UPC * c + u for u in range(UPC)]
        wint = np.ascontiguousarray(
            (np.stack([w_in[g].T.reshape(KC, 128, N) for g in us])
             .transpose(2, 0, 1, 3).reshape(128, UPC * KC * N)
             * np.float32(W_SCALE)).astype(ml_dtypes.float8_e4m3fn))
        spc = np.ascontiguousarray(sp[:, c * 128:(c + 1) * 128])
        in_maps.append({"kspt": kspt, "wint": wint, "spc": spc})
    return in_maps


# ---------------------------------------------------------------- fallback
def _reference_host(sp, w_in, w_rec, unit_w, cw1, cb1, cw2, cb2):
    """Exact sequential evaluation (used only if any spike fires)."""
    m = np.float32(DT * TAU_MEM_INV)
    bsyn = np.float32(1.0 - DT * TAU_SYN_INV)
    outs = np.zeros((U, T, N), np.float32)
    for uu in range(U):
        z = np.zeros(N, np.float32)
        v = np.full(N, V_LEAK, np.float32)
        i = np.zeros(N, np.float32)
        for t in range(T):
            vd = v + m * ((V_LEAK - v) + i)
            idec = i * bsyn
            zn = (vd - V_TH > 0).astype(np.float32)
            vn = (1 - zn) * vd + zn * V_RESET
            i = idec + sp[t] @ w_in[uu].T + z @ w_rec[uu].T
            z, v = zn, vn
            outs[uu, t] = zn
    act = outs.mean(axis=1)
    h = np.maximum(act.reshape(-1) @ cw1.T + cb1, 0).astype(np.float32)
    probs = (1.0 / (1.0 + np.exp(-(h @ cw2.T + cb2)))).reshape(U, U)
    conn = (_u42() < probs).astype(np.float32)
    routed = np.einsum('ij,itn->tjn', conn, outs)
    applied = np.einsum('tjn,jnm->tjm', routed, unit_w)
    return (applied.mean(axis=1) + DIRECT_WEIGHT * sp).astype(np.float32)


# ---------------------------------------------------------------- entry
def kernel(input_spikes, w_in, w_rec, unit_w, cw1, cb1, cw2, cb2,
           **_unused):
    sp = np.ascontiguousarray(np.asarray(input_spikes, np.float32))
    w_in = np.asarray(w_in, np.float32)

    nc = _build_graph()
    in_maps = _prep_in_maps(sp, w_in)
    res = run_bass_kernel_spmd(nc, in_maps, core_ids=list(range(NCORES)))
    maxdrift = max(float(np.asarray(res.results[c]["zsum"]).reshape(-1)[0])
                   for c in range(NCORES))
    if maxdrift > DET_TH:
        # A spike may fire: the linearized fast path is invalid -> exact
        # host evaluation (never hit with the benchmark weight scales).
        return _reference_host(
            sp, w_in, np.asarray(w_rec, np.float32),
            np.asarray(unit_w, np.float32), np.asarray(cw1, np.float32),
            np.asarray(cb1, np.float32), np.asarray(cw2, np.float32),
            np.asarray(cb2, np.float32))
    out = np.concatenate(
        [np.asarray(res.results[c]["out"], np.float32)
         for c in range(NCORES)], axis=1)
    return np.ascontiguousarray(out)


if __name__ == "__main__":
    d = np.load("inputs.npz")
    got = kernel(**{k: d[k] for k in d.files})
    ref = np.load("golden.npy")
    err = np.abs(got - ref).max()
    denom = max(np.abs(ref).max(), 1e-9)
    print("abs err:", err, "rel:", err / denom)
